# revision 1
# baseline (speedup 1.0000x reference)
"""Pairwise-distance + global max normalize kernel for trn2, 8 cores.

Problem (hardcoded): x [4, 4096, 64] f32 ->
    out[b] = cdist(x[b], x[b]) / global_max, diag set to 1.0.
    (The reference normalizes (d - dmin)/(dmax - dmin); dmin is the
    diagonal of cdist-via-matmul-identity which rounds to ~0/tiny-neg,
    so dmin = 0: worst-case disagreement < 6e-4 relative; measured
    end-to-end error 1.2e-4, dominated by the f32r matmul mode.)

Structure per core (SPMD, core c -> batch c//2, row-half c%2):
  - d2 tiles are produced directly by one K=66 matmul: stationary rows
    0:64 = -2*x_rows^T, row 64 = sq_rows, row 65 = ones; moving rows
    0:64 = x_cols^T, row 64 = ones, row 65 = sq_cols.  Operands are
    float32r (single-pass PE mode, ~2-3x faster than exact fp32;
    costs ~1e-4 relative error, well under tolerance).
  - pass A (max scan): only unique pairs are scanned.  Globally the 4
    batches decompose into 40 [1024x1024] quarter-block pairs
    ((q,q) x4 + (q,r) q<r x6 per batch); each core scans 5 of them
    (same shapes on every core -> SPMD-uniform), reduce_max on DVE at
    [128,1024] width from PSUM.
  - AllReduce(max) of the per-partition maxima across the 8 cores.
  - pass B: recompute d2 for this core's [2048,4096] output block,
    out = Sqrt(d2 * (1/max_d2)) on ACT (scale is per-partition SBUF
    operand), DMA to DRAM.  Diagonal d2 can round tiny-negative ->
    Sqrt NaN there; the host overwrites the diagonal with exactly 1.0
    (as the reference does).  Off-diagonal d2 >= ~16 for this data.
"""

import numpy as np

B = 4
N = 4096
D = 64
NCORES = 8
ROWS = N // 2  # 2048 rows per core
K = D + 2  # 66
PT = 128
FT = 512  # one fp32 PSUM bank
WT = 1024  # working tile width (2 banks)
RT = ROWS // PT  # 16 row tiles (pass B)
CG = N // WT  # 4 col groups (pass B)
Q = 1024  # quarter-block size (pass A)
NBLK = 5  # pair-blocks per core
QRT = Q // PT  # 8 row tiles per pair-block

# 40 unique quarter-block pairs (batch, qa, qb); core c takes [5c:5c+5].
PAIR_BLOCKS = [
    (b, qa, qb) for b in range(B) for qa in range(4) for qb in range(qa, 4)
]
assert len(PAIR_BLOCKS) == NCORES * NBLK

_CACHE = {}
LAST_RESULTS = None


def _build_nc():
    import concourse.bacc as bacc
    import concourse.tile as tile
    from concourse import mybir

    f32 = mybir.dt.float32
    f32r = mybir.dt.float32r
    nc = bacc.Bacc(None, target_bir_lowering=False)

    kxm = nc.dram_tensor("kxm", [K, ROWS], f32r, kind="ExternalInput")
    kxn = nc.dram_tensor("kxn", [K, N], f32r, kind="ExternalInput")
    pa = nc.dram_tensor("pa", [K, NBLK * Q], f32r, kind="ExternalInput")
    pb = nc.dram_tensor("pb", [K, NBLK * Q], f32r, kind="ExternalInput")
    out = nc.dram_tensor("out", [ROWS, N], f32, kind="ExternalOutput")

    with tile.TileContext(nc) as tc:
        with (
            tc.tile_pool(name="singles", bufs=1) as singles,
            tc.tile_pool(name="outp", bufs=4) as outp,
            tc.tile_pool(name="ps", bufs=2, space="PSUM") as psp,
            tc.tile_pool(name="psS", bufs=1, space="PSUM") as psS,
            tc.tile_pool(name="dram", bufs=2, space="DRAM") as dram,
        ):
            pa_s = singles.tile([K, NBLK * Q], f32r)
            pb_s = singles.tile([K, NBLK * Q], f32r)
            for q in range(NBLK):
                nc.sync.dma_start(out=pa_s[:, q * Q : (q + 1) * Q], in_=pa[:, q * Q : (q + 1) * Q])
                nc.sync.dma_start(out=pb_s[:, q * Q : (q + 1) * Q], in_=pb[:, q * Q : (q + 1) * Q])
            kxm_s = singles.tile([K, ROWS], f32r)
            nc.scalar.dma_start(out=kxm_s[:], in_=kxm[:])
            kxn_s = singles.tile([K, N], f32r)
            nc.scalar.dma_start(out=kxn_s[:], in_=kxn[:])

            # ---- pass A: max(d2) over this core's 5 unique pair-blocks ----
            stats = singles.tile([PT, NBLK * QRT], f32)
            for q in range(NBLK):
                for rt in range(QRT):
                    ps = psp.tile([PT, WT], f32, tag="ps")
                    for j in range(WT // FT):
                        nc.tensor.matmul(
                            ps[:, j * FT : (j + 1) * FT],
                            pa_s[:, q * Q + rt * PT : q * Q + (rt + 1) * PT],
                            pb_s[:, q * Q + j * FT : q * Q + (j + 1) * FT],
                            start=True,
                            stop=True,
                        )
                    idx = q * QRT + rt
                    nc.vector.reduce_max(
                        out=stats[:, idx : idx + 1],
                        in_=ps[:],
                        axis=mybir.AxisListType.X,
                    )
            loc = singles.tile([PT, 1], f32)
            nc.vector.reduce_max(out=loc[:], in_=stats[:], axis=mybir.AxisListType.X)

            # ---- all-reduce (max) across the 8 cores ----
            inb = dram.tile([1, PT], f32)
            outb = dram.tile([1, PT], f32)
            nc.gpsimd.dma_start(out=inb[:], in_=loc[:])
            nc.gpsimd.collective_compute(
                "AllReduce",
                mybir.AluOpType.max,
                replica_groups=[list(range(NCORES))],
                ins=[inb[:].opt()],
                outs=[outb[:].opt()],
            )
            mxrow = singles.tile([1, PT], f32)
            nc.gpsimd.dma_start(out=mxrow[:], in_=outb[:])
            mx = singles.tile([1, 1], f32)
            nc.vector.reduce_max(out=mx[:], in_=mxrow[:], axis=mybir.AxisListType.X)

            # mx = max(d2) = dmax^2; scale = 1/mx, broadcast via K=1 matmul.
            s2 = singles.tile([1, 1], f32)
            nc.vector.reciprocal(out=s2[:], in_=mx[:])
            ones = singles.tile([1, PT], f32)
            nc.vector.memset(ones[:], 1.0)
            ps_s2 = psS.tile([PT, 1], f32, tag="psS")
            nc.tensor.matmul(ps_s2[:], ones[:], s2[:], start=True, stop=True)
            s2b = singles.tile([PT, 1], f32)
            nc.scalar.copy(out=s2b[:], in_=ps_s2[:])

            # ---- pass B: recompute d2, out = Sqrt(d2/mx), store ----
            for rt in range(RT):
                for cg in range(CG):
                    ps = psp.tile([PT, WT], f32, tag="ps")
                    for j in range(WT // FT):
                        nc.tensor.matmul(
                            ps[:, j * FT : (j + 1) * FT],
                            kxm_s[:, rt * PT : (rt + 1) * PT],
                            kxn_s[:, (cg * 2 + j) * FT : (cg * 2 + j + 1) * FT],
                            start=True,
                            stop=True,
                        )
                    o = outp.tile([PT, WT], f32, tag="o")
                    nc.scalar.activation(
                        out=o[:],
                        in_=ps[:],
                        func=mybir.ActivationFunctionType.Sqrt,
                        bias=0.0,
                        scale=s2b[:],
                    )
                    nc.sync.dma_start(
                        out=out[rt * PT : (rt + 1) * PT, cg * WT : (cg + 1) * WT],
                        in_=o[:],
                    )

    nc.finalize()
    return nc


def _get_nc():
    if "nc" not in _CACHE:
        _CACHE["nc"] = _build_nc()
    return _CACHE["nc"]


def _lhs_block(xblk, sqblk):
    """Stationary-operand layout [K, n]: -2x^T / sq / ones."""
    n = xblk.shape[0]
    m = np.empty((K, n), dtype=np.float32)
    m[:D] = (-2.0 * xblk).T
    m[D] = sqblk
    m[D + 1] = 1.0
    return m


def _rhs_block(xblk, sqblk):
    """Moving-operand layout [K, n]: x^T / ones / sq."""
    n = xblk.shape[0]
    m = np.empty((K, n), dtype=np.float32)
    m[:D] = xblk.T
    m[D] = 1.0
    m[D + 1] = sqblk
    return m


def kernel(x):
    global LAST_RESULTS
    from concourse.bass_utils import run_bass_kernel_spmd

    x = np.asarray(x, dtype=np.float32)
    assert x.shape == (B, N, D), x.shape

    sqs = [(x[b].astype(np.float64) ** 2).sum(-1).astype(np.float32) for b in range(B)]

    in_maps = []
    for c in range(NCORES):
        b, h = divmod(c, 2)
        xb, sq = x[b], sqs[b]
        kxm = _lhs_block(xb[h * ROWS : (h + 1) * ROWS], sq[h * ROWS : (h + 1) * ROWS])
        kxn = _rhs_block(xb, sq)
        pas, pbs = [], []
        for (bb, qa, qb) in PAIR_BLOCKS[c * NBLK : (c + 1) * NBLK]:
            xq, sqq = x[bb], sqs[bb]
            pas.append(_lhs_block(xq[qa * Q : (qa + 1) * Q], sqq[qa * Q : (qa + 1) * Q]))
            pbs.append(_rhs_block(xq[qb * Q : (qb + 1) * Q], sqq[qb * Q : (qb + 1) * Q]))
        pa = np.ascontiguousarray(np.concatenate(pas, axis=1))
        pb = np.ascontiguousarray(np.concatenate(pbs, axis=1))
        in_maps.append(
            {
                "kxm": np.ascontiguousarray(kxm),
                "kxn": np.ascontiguousarray(kxn),
                "pa": pa,
                "pb": pb,
            }
        )

    nc = _get_nc()
    res = run_bass_kernel_spmd(nc, in_maps, core_ids=list(range(NCORES)))
    LAST_RESULTS = res

    out = np.empty((B, N, N), dtype=np.float32)
    for c in range(NCORES):
        b, h = divmod(c, 2)
        out[b, h * ROWS : (h + 1) * ROWS, :] = res.results[c]["out"]
    di = np.arange(N)
    out[:, di, di] = 1.0
    return out



# revision 4
# speedup vs baseline: 1.9470x; 1.9470x over previous
"""Pairwise-distance + global max normalize kernel for trn2, 8 cores.

Problem (hardcoded): x [4, 4096, 64] f32 ->
    out[b] = cdist(x[b], x[b]) / global_max_dist, diag = 1.0.
    (Reference computes (d - dmin)/(dmax - dmin); dmin is ~0, see baseline
    notes; neglecting it costs ~1e-4 relative.)

Symmetry-aware triangle sharding, one compute pass + cheap rescale:
  - The [N,N] distance matrix is symmetric: only the upper block-triangle
    is computed/written on-device (~54% of elements); the host mirrors
    the lower triangle during unsharding (and sets the diagonal to 1.0).
  - Row-tiles of 128 rows are grouped in width classes k=0..15; the tile
    for class k covers cols [256k, 4096), width W_k = 4096-256k.  Each
    batch has 32 row-tiles (2 per class); 2 cores per batch take the
    even/odd ones -> every core gets exactly one tile per class: an
    SPMD-uniform shape schedule.
  - Phase 1 (per tile): PE matmuls d2 into PSUM via the K=66 identity
    (bf16 operands: [-2x^T; sq; 1] x [x^T; 1; sq]).  For wide tiles
    (k<8) ACT computes u = Sqrt(d2 + 0.01) into an SBUF fp16 store (the
    +0.01 keeps the diagonal's tiny-negative d2 from producing NaN that
    would poison the max scan; host discards the diagonal anyway) and
    DVE accumulates a running fp16 max (2x DVE mode).  Narrow tiles
    (k>=8) are max-reduced by DVE straight from PSUM and recomputed in
    phase 2.
  - The per-core max is folded across partitions (gpsimd
    partition_all_reduce), AllGather'd across the 8 cores as one f32
    scalar (AllGather, 15us, vs AllReduce's 1.875x multiplier), reduced,
    inverted (s = 1/dmax), and partition_broadcast.
  - Phase 2: wide tiles are scaled in place by s on DVE (4x fp16
    tensor_scalar mode) and DMA'd out as fp16; narrow tiles are
    recomputed by PE and written via fused ACT Sqrt(s^2*d2 + s^2*0.01).
  - fp16 output + bf16 inputs cost ~1e-3 relative error combined,
    far under the 2e-2 gate, and halve the dominant DMA traffic.
"""

import numpy as np

B = 4
N = 4096
D = 64
NCORES = 8
K = D + 2  # 66
PT = 128
FT = 512  # one f32 PSUM bank
CT = 2048  # PSUM working tile (4 banks)
NK = 16  # width classes
WIDTHS = [N - 256 * k for k in range(NK)]  # 4096, 3840, ..., 256
SET_A = set(range(0, 8))   # wide tiles: sqrt in phase 1, u kept in SBUF
SET_B = set(range(8, 16))  # narrow tiles: PSUM max scan, recomputed in phase 2

# Phase-1 order: narrow (set B) tiles first so PE can start on the
# back-loaded kxn chunks; wide (set A) tiles interleaved from position 3
# so ACT gets busy as soon as its kxn cols land.
SEQ1 = [15, 14, 13, 7, 12, 6, 11, 5, 10, 4, 9, 3, 8, 2, 1, 0]
assert sorted(SEQ1) == list(range(NK))
# Packed output/kxm column offsets follow SEQ1 order.
OFF = {}
_o = 0
for _k in SEQ1:
    OFF[_k] = _o
    _o += WIDTHS[_k]
TOTW = _o  # 34816
UOFF = {}  # u_store offsets for set A tiles
_o = 0
for _k in SEQ1:
    if _k in SET_A:
        UOFF[_k] = _o
        _o += WIDTHS[_k]
UTOT = _o  # 25600

# Phase-2 order: set A (instant DVE scale) first to feed DMA, then set B.
SEQ2 = [k for k in SEQ1 if k in SET_A] + [k for k in SEQ1 if k in SET_B]

SQRT_BIAS = 0.01

_CACHE = {}
LAST_RESULTS = None


def _build_nc():
    import concourse.bacc as bacc
    import concourse.tile as tile
    from concourse import bass_isa, mybir

    f32 = mybir.dt.float32
    f16 = mybir.dt.float16
    bf16 = mybir.dt.bfloat16
    nc = bacc.Bacc(None, target_bir_lowering=False)

    kxm = nc.dram_tensor("kxm", [K, NK * PT], bf16, kind="ExternalInput")
    kxn = nc.dram_tensor("kxn", [K, N], bf16, kind="ExternalInput")
    out = nc.dram_tensor("out", [PT, TOTW], f16, kind="ExternalOutput")

    with tile.TileContext(nc) as tc:
        with (
            tc.tile_pool(name="singles", bufs=1) as singles,
            tc.tile_pool(name="stage", bufs=3) as stage,
            tc.tile_pool(name="ps", bufs=2, space="PSUM") as psp,
            tc.tile_pool(name="dram", bufs=2, space="DRAM") as dram,
        ):
            # ---- input loads: kxm in 2 chunks (SEQ1-packed cols), kxn in
            # 512-col chunks back to front (narrow tiles need the tail) ----
            kxm_s = singles.tile([K, NK * PT], bf16)
            nc.sync.dma_start(out=kxm_s[:, : 8 * PT], in_=kxm[:, : 8 * PT])
            nc.sync.dma_start(out=kxm_s[:, 8 * PT :], in_=kxm[:, 8 * PT :])
            kxn_s = singles.tile([K, N], bf16)
            NCH = 8
            for j in range(NCH):
                lo, hi = N - 512 * (j + 1), N - 512 * j
                nc.sync.dma_start(out=kxn_s[:, lo:hi], in_=kxn[:, lo:hi])

            # ---- warmups: Sqrt act table load + acc init, off critical path
            u_store = singles.tile([PT, UTOT], f16)
            acc = singles.tile([PT, CT], f16)
            nc.gpsimd.memset(acc[:], 0.0)
            dum = singles.tile([PT, 1], f32)
            nc.gpsimd.memset(dum[:], 1.0)
            beps = singles.tile([PT, 1], f32)
            nc.gpsimd.memset(beps[:], SQRT_BIAS)
            dum16 = singles.tile([PT, 1], f16)
            nc.scalar.activation(
                out=dum16[:], in_=dum[:],
                func=mybir.ActivationFunctionType.Sqrt, bias=beps[:], scale=1.0,
            )

            stats = singles.tile([PT, 16], f16)
            nstat = 0

            # ---- phase 1: d2 tiles; sqrt+store (A) or PSUM max-scan (B) ----
            for pos, k in enumerate(SEQ1):
                w = WIDTHS[k]
                c0 = N - w  # first col
                for ch0 in range(0, w, CT):
                    cw = min(CT, w - ch0)
                    ps = psp.tile([PT, CT], f32, tag="ps")
                    for j0 in range(0, cw, FT):
                        jw = min(FT, cw - j0)
                        nc.tensor.matmul(
                            ps[:, j0 : j0 + jw],
                            kxm_s[:, pos * PT : (pos + 1) * PT],
                            kxn_s[:, c0 + ch0 + j0 : c0 + ch0 + j0 + jw],
                            start=True,
                            stop=True,
                        )
                    if k in SET_A:
                        uo = UOFF[k] + ch0
                        nc.scalar.activation(
                            out=u_store[:, uo : uo + cw],
                            in_=ps[:, :cw],
                            func=mybir.ActivationFunctionType.Sqrt,
                            bias=beps[:],
                            scale=1.0,
                        )
                        nc.vector.tensor_tensor(
                            out=acc[:, :cw],
                            in0=acc[:, :cw],
                            in1=u_store[:, uo : uo + cw],
                            op=mybir.AluOpType.max,
                        )
                    else:
                        nc.vector.reduce_max(
                            out=stats[:, nstat : nstat + 1],
                            in_=ps[:, :cw],
                            axis=mybir.AxisListType.X,
                        )
                        nstat += 1

            # acc holds max of u = sqrt(d2 + eps); stats holds max of raw d2.
            # Convert the d2 stats to u-scale: sqrt(max_d2 + eps) via ACT.
            d2m = singles.tile([PT, 1], f16)
            nc.vector.reduce_max(
                out=d2m[:], in_=stats[:, :nstat], axis=mybir.AxisListType.X
            )
            d2mf = singles.tile([PT, 1], f32)
            nc.vector.tensor_scalar(
                out=d2mf[:], in0=d2m[:], scalar1=1.0, scalar2=None,
                op0=mybir.AluOpType.mult,
            )
            um_b = singles.tile([PT, 1], f16)
            nc.scalar.activation(
                out=um_b[:], in_=d2mf[:],
                func=mybir.ActivationFunctionType.Sqrt, bias=beps[:], scale=1.0,
            )
            um_a = singles.tile([PT, 1], f16)
            nc.vector.reduce_max(out=um_a[:], in_=acc[:], axis=mybir.AxisListType.X)
            loc = singles.tile([PT, 1], f32)
            nc.vector.tensor_tensor(
                out=loc[:], in0=um_a[:], in1=um_b[:], op=mybir.AluOpType.max
            )

            # ---- cross-partition + cross-core max of u; s = 1/max ----
            par = singles.tile([PT, 1], f32)
            nc.gpsimd.partition_all_reduce(
                out_ap=par[:], in_ap=loc[:], channels=PT,
                reduce_op=bass_isa.ReduceOp.max,
            )
            inb = dram.tile([1, 1], f32)
            outb = dram.tile([1, NCORES], f32)
            nc.sync.dma_start(out=inb[:], in_=par[0:1, :])
            nc.gpsimd.collective_compute(
                "AllGather",
                mybir.AluOpType.bypass,
                replica_groups=[list(range(NCORES))],
                ins=[inb[:].opt()],
                outs=[outb[:].opt()],
            )
            g = singles.tile([1, NCORES], f32)
            nc.sync.dma_start(out=g[:], in_=outb[:])
            mx = singles.tile([1, 1], f32)
            nc.vector.reduce_max(out=mx[:], in_=g[:], axis=mybir.AxisListType.X)
            s1 = singles.tile([1, 1], f32)
            nc.vector.reciprocal(out=s1[:], in_=mx[:])
            s2 = singles.tile([1, 1], f32)
            nc.vector.tensor_tensor(
                out=s2[:], in0=s1[:], in1=s1[:], op=mybir.AluOpType.mult
            )
            sb = singles.tile([PT, 1], f32)
            nc.gpsimd.partition_broadcast(out_ap=sb[:], in_ap=s1[:])
            s2b = singles.tile([PT, 1], f32)
            nc.gpsimd.partition_broadcast(out_ap=s2b[:], in_ap=s2[:])
            b2 = singles.tile([PT, 1], f32)
            nc.vector.tensor_scalar(
                out=b2[:], in0=s2b[:], scalar1=SQRT_BIAS, scalar2=None,
                op0=mybir.AluOpType.mult,
            )

            # ---- phase 2: scale + store ----
            for k in SEQ2:
                w = WIDTHS[k]
                c0 = N - w
                o = OFF[k]
                if k in SET_A:
                    uo = UOFF[k]
                    nc.vector.tensor_scalar(
                        out=u_store[:, uo : uo + w],
                        in0=u_store[:, uo : uo + w],
                        scalar1=sb[:],
                        scalar2=None,
                        op0=mybir.AluOpType.mult,
                    )
                    nc.sync.dma_start(
                        out=out[:, o : o + w], in_=u_store[:, uo : uo + w]
                    )
                else:
                    pos = SEQ1.index(k)
                    ps = psp.tile([PT, CT], f32, tag="ps")
                    for j0 in range(0, w, FT):
                        jw = min(FT, w - j0)
                        nc.tensor.matmul(
                            ps[:, j0 : j0 + jw],
                            kxm_s[:, pos * PT : (pos + 1) * PT],
                            kxn_s[:, c0 + j0 : c0 + j0 + jw],
                            start=True,
                            stop=True,
                        )
                    st = stage.tile([PT, CT], f16, tag="st")
                    nc.scalar.activation(
                        out=st[:, :w],
                        in_=ps[:, :w],
                        func=mybir.ActivationFunctionType.Sqrt,
                        bias=b2[:],
                        scale=s2b[:],
                    )
                    nc.sync.dma_start(out=out[:, o : o + w], in_=st[:, :w])

    nc.finalize()
    return nc


def _get_nc():
    if "nc" not in _CACHE:
        _CACHE["nc"] = _build_nc()
    return _CACHE["nc"]


def kernel(x):
    global LAST_RESULTS
    import ml_dtypes
    from concourse.bass_utils import run_bass_kernel_spmd

    bf16 = ml_dtypes.bfloat16
    x = np.asarray(x, dtype=np.float32)
    assert x.shape == (B, N, D), x.shape

    in_maps = []
    for c in range(NCORES):
        b, p = divmod(c, 2)
        xb = x[b]
        sq = (xb.astype(np.float64) ** 2).sum(-1).astype(np.float32)
        # kxn: moving operand [K, N] = [x^T; 1; sq]
        kxn = np.empty((K, N), dtype=np.float32)
        kxn[:D] = xb.T
        kxn[D] = 1.0
        kxn[D + 1] = sq
        # kxm: stationary operand [K, 16*128], col block pos <-> class SEQ1[pos]
        kxm = np.empty((K, NK * PT), dtype=np.float32)
        for pos, k in enumerate(SEQ1):
            r0 = PT * (2 * k + p)
            rows = slice(r0, r0 + PT)
            kxm[:D, pos * PT : (pos + 1) * PT] = (-2.0 * xb[rows]).T
            kxm[D, pos * PT : (pos + 1) * PT] = sq[rows]
            kxm[D + 1, pos * PT : (pos + 1) * PT] = 1.0
        in_maps.append(
            {
                "kxm": np.ascontiguousarray(kxm.astype(bf16)),
                "kxn": np.ascontiguousarray(kxn.astype(bf16)),
            }
        )

    nc = _get_nc()
    res = run_bass_kernel_spmd(nc, in_maps, core_ids=list(range(NCORES)))
    LAST_RESULTS = res

    out = np.zeros((B, N, N), dtype=np.float32)
    for c in range(NCORES):
        b, p = divmod(c, 2)
        buf = np.asarray(res.results[c]["out"])
        for k in range(NK):
            w = WIDTHS[k]
            r0 = PT * (2 * k + p)
            out[b, r0 : r0 + PT, N - w :] = buf[:, OFF[k] : OFF[k] + w]
    # mirror the strict upper triangle into the lower one; diagonal = 1.0
    for b in range(B):
        m = np.triu(out[b], 1)
        out[b] = m + m.T
        np.fill_diagonal(out[b], 1.0)
    return out


# revision 8
# speedup vs baseline: 2.1871x; 1.1233x over previous
"""Pairwise-distance + global max normalize kernel for trn2, 8 cores.

Problem (hardcoded): x [4, 4096, 64] f32 ->
    out[b] = cdist(x[b], x[b]) / global_max_dist, diag = 1.0.
    (Reference computes (d - dmin)/(dmax - dmin); dmin is ~0, see baseline
    notes; neglecting it costs ~1e-4 relative.)

Symmetry-aware triangle sharding, one compute pass + cheap rescale:
  - The [N,N] distance matrix is symmetric: only the upper block-triangle
    is computed/written on-device (~54% of elements); the host mirrors
    the lower triangle during unsharding (and sets the diagonal to 1.0).
  - Row-tiles of 128 rows are grouped in width classes k=0..15; the tile
    for class k covers cols [256k, 4096), width W_k = 4096-256k.  Each
    batch has 32 row-tiles (2 per class); 2 cores per batch take the
    even/odd ones -> every core gets exactly one tile per class: an
    SPMD-uniform shape schedule.
  - Phase 1 runs an explicit chunk schedule: PE matmuls d2 into PSUM via
    the K=66 identity (bf16 operands: [-2x^T; sq; 1] x [x^T; 1; sq]).
    Wide tiles (set A, k<7): ACT computes u = Sqrt(d2 + 0.01) into an
    SBUF fp16 store (the +0.01 keeps the diagonal's tiny-negative d2
    from producing NaN that would poison the max scan; the host
    discards the diagonal anyway); DVE scans u with an in-place
    identity tensor_scalar whose fused accumulator takes the
    per-partition max (4x fp16 mode, ~0.29 ns/elem).  Narrow tiles
    (set B) are max-reduced by DVE straight from PSUM (d2 domain) and
    recomputed in phase 2.  Chunks that only need the back half of kxn
    (all set B + set-A back chunks) run first so compute starts as soon
    as the first input chunks land; set-A front chunks run last.
  - The two max domains are merged (u_max vs sqrt(d2_max + 0.01)), the
    per-core max is folded across partitions (gpsimd
    partition_all_reduce), AllGather'd across the 8 cores as one f32
    scalar (AllGather, 15us, vs AllReduce's 1.875x multiplier), reduced,
    inverted (s = 1/dmax), and partition_broadcast.
  - Phase 2: set A is scaled in place by s on DVE (4x fp16
    tensor_scalar mode) and DMA'd out per chunk; set B is recomputed by
    PE (prefetched during the collective) and written via fused ACT
    Sqrt(s^2*d2 + s^2*0.01) through staging tiles.  Set-B DMAs ride the
    Pool SWDGE queue so they cannot head-of-line block behind set-A
    DMAs on the SP queue.
  - fp16 output + bf16 inputs cost ~2.7e-3 relative error combined,
    far under the 2e-2 gate, and halve the dominant DMA traffic.
"""

import numpy as np

B = 4
N = 4096
D = 64
NCORES = 8
K = D + 2  # 66
PT = 128
FT = 512  # one f32 PSUM bank
CT = 2048  # PSUM working tile (4 banks)
NK = 16  # width classes
WIDTHS = [N - 256 * k for k in range(NK)]  # 4096, 3840, ..., 256
SET_A = set(range(0, 7))   # wide tiles: sqrt in phase 1, u kept in SBUF
SET_B = set(range(7, 16))  # narrow tiles: PSUM max scan, recomputed in phase 2

# kxm column-block order (block pos p holds tile TILE_POS[p]'s rows);
# the first 8 blocks are the tiles that start phase 1.
TILE_POS = [15, 0, 14, 1, 13, 2, 12, 3, 11, 4, 10, 5, 9, 6, 8, 7]
POS_OF = {k: p for p, k in enumerate(TILE_POS)}

# Phase-1 chunk schedule (tile k, chunk col offset ch0).  Chunks needing
# only kxn cols >= 2048 run first (interleaving set-B tiles with set-A
# back chunks); set-A front chunks + the two low set-B chunks run last.
CHUNKS_P1 = [
    (15, 0), (0, 2048), (14, 0), (1, 2048), (13, 0), (2, 2048),
    (12, 0), (3, 2048), (11, 0), (4, 2048), (10, 0), (5, 2048),
    (9, 0), (6, 2048), (7, 2048),
    (0, 0), (7, 0), (1, 0), (2, 0), (8, 0), (3, 0), (4, 0), (5, 0), (6, 0),
]
assert sorted(CHUNKS_P1) == sorted(
    (k, c) for k in range(NK) for c in range(0, WIDTHS[k], CT)
)

# Output packing: tile k at column offset OFF[k] (natural order).
OFF = {}
_o = 0
for _k in range(NK):
    OFF[_k] = _o
    _o += WIDTHS[_k]
TOTW = _o  # 34816
UOFF = {}  # u_store offsets for set A tiles
_o = 0
for _k in sorted(SET_A):
    UOFF[_k] = _o
    _o += WIDTHS[_k]
UTOT = _o  # 23296

# Phase-2 set-B order: widest first so ACT/DMA start early.
SEQ2_B = [8, 7, 9, 10, 11, 12, 13, 14, 15]

SQRT_BIAS = 0.01

_CACHE = {}
LAST_RESULTS = None


def _build_nc():
    import concourse.bacc as bacc
    import concourse.tile as tile
    from concourse import bass_isa, mybir

    f32 = mybir.dt.float32
    f16 = mybir.dt.float16
    bf16 = mybir.dt.bfloat16
    nc = bacc.Bacc(None, target_bir_lowering=False)

    kxm = nc.dram_tensor("kxm", [K, NK * PT], bf16, kind="ExternalInput")
    kxn = nc.dram_tensor("kxn", [K, N], bf16, kind="ExternalInput")
    out = nc.dram_tensor("out", [PT, TOTW], f16, kind="ExternalOutput")

    with tile.TileContext(nc) as tc:
        with (
            tc.tile_pool(name="singles", bufs=1) as singles,
            tc.tile_pool(name="stage", bufs=6) as stage,
            tc.tile_pool(name="ps", bufs=2, space="PSUM") as psp,
            tc.tile_pool(name="dram", bufs=2, space="DRAM") as dram,
        ):
            # ---- input loads, split across the SP (HWDGE) and Pool (SWDGE)
            # queues.  kxm block 1 and the back half of kxn gate the start;
            # the front half of kxn is only needed by the late chunks.
            kxm_s = singles.tile([K, NK * PT], bf16)
            kxn_s = singles.tile([K, N], bf16)
            nc.sync.dma_start(out=kxm_s[:, : 8 * PT], in_=kxm[:, : 8 * PT])
            nc.gpsimd.dma_start(out=kxn_s[:, 2048:3072], in_=kxn[:, 2048:3072])
            nc.sync.dma_start(out=kxn_s[:, 3072:4096], in_=kxn[:, 3072:4096])
            nc.gpsimd.dma_start(out=kxm_s[:, 8 * PT :], in_=kxm[:, 8 * PT :])
            nc.sync.dma_start(out=kxn_s[:, 1024:2048], in_=kxn[:, 1024:2048])
            nc.gpsimd.dma_start(out=kxn_s[:, 0:1024], in_=kxn[:, 0:1024])

            # ---- warmups: Sqrt act table load + bias const, off critical path
            u_store = singles.tile([PT, UTOT], f16)
            dum = singles.tile([PT, 1], f32)
            nc.vector.memset(dum[:], 1.0)
            beps = singles.tile([PT, 1], f32)
            nc.vector.memset(beps[:], SQRT_BIAS)
            dum16 = singles.tile([PT, 1], f16)
            nc.scalar.activation(
                out=dum16[:], in_=dum[:],
                func=mybir.ActivationFunctionType.Sqrt, bias=beps[:], scale=1.0,
            )

            # stats: cols [0, na) = set-A u-max accums, [16, 16+nb) = set-B
            # d2-max reduces.
            stats = singles.tile([PT, 32], f32)
            na = 0
            nb = 0

            # ---- phase 1 ----
            for k, ch0 in CHUNKS_P1:
                w = WIDTHS[k]
                c0 = N - w
                cw = min(CT, w - ch0)
                pos = POS_OF[k]
                ps = psp.tile([PT, CT], f32, tag="ps")
                for j0 in range(0, cw, FT):
                    jw = min(FT, cw - j0)
                    nc.tensor.matmul(
                        ps[:, j0 : j0 + jw],
                        kxm_s[:, pos * PT : (pos + 1) * PT],
                        kxn_s[:, c0 + ch0 + j0 : c0 + ch0 + j0 + jw],
                        start=True,
                        stop=True,
                    )
                if k in SET_A:
                    uo = UOFF[k] + ch0
                    nc.scalar.activation(
                        out=u_store[:, uo : uo + cw],
                        in_=ps[:, :cw],
                        func=mybir.ActivationFunctionType.Sqrt,
                        bias=beps[:],
                        scale=1.0,
                    )
                    nc.vector.tensor_scalar(
                        out=u_store[:, uo : uo + cw],
                        in0=u_store[:, uo : uo + cw],
                        scalar1=1.0,
                        scalar2=None,
                        op0=mybir.AluOpType.mult,
                        op1=mybir.AluOpType.max,
                        accum_out=stats[:, na : na + 1],
                    )
                    na += 1
                else:
                    nc.vector.reduce_max(
                        out=stats[:, 16 + nb : 17 + nb],
                        in_=ps[:, :cw],
                        axis=mybir.AxisListType.X,
                    )
                    nb += 1

            # Merge domains: loc = max(max_u, sqrt(max_d2 + eps)).
            redb = singles.tile([PT, 1], f32)
            nc.vector.reduce_max(
                out=redb[:], in_=stats[:, 16 : 16 + nb], axis=mybir.AxisListType.X
            )
            um_b = singles.tile([PT, 1], f32)
            nc.scalar.activation(
                out=um_b[:], in_=redb[:],
                func=mybir.ActivationFunctionType.Sqrt, bias=beps[:], scale=1.0,
            )
            reda = singles.tile([PT, 1], f32)
            nc.vector.reduce_max(
                out=reda[:], in_=stats[:, :na], axis=mybir.AxisListType.X
            )
            loc = singles.tile([PT, 1], f32)
            nc.vector.tensor_tensor(
                out=loc[:], in0=reda[:], in1=um_b[:], op=mybir.AluOpType.max
            )

            # ---- cross-partition + cross-core max of u; s = 1/max ----
            par = singles.tile([PT, 1], f32)
            nc.gpsimd.partition_all_reduce(
                out_ap=par[:], in_ap=loc[:], channels=PT,
                reduce_op=bass_isa.ReduceOp.max,
            )
            inb = dram.tile([1, 1], f32)
            outb = dram.tile([1, NCORES], f32)
            nc.sync.dma_start(out=inb[:], in_=par[0:1, :])
            nc.gpsimd.collective_compute(
                "AllGather",
                mybir.AluOpType.bypass,
                replica_groups=[list(range(NCORES))],
                ins=[inb[:].opt()],
                outs=[outb[:].opt()],
            )
            g = singles.tile([1, NCORES], f32)
            nc.sync.dma_start(out=g[:], in_=outb[:])
            mx = singles.tile([1, 1], f32)
            nc.vector.reduce_max(out=mx[:], in_=g[:], axis=mybir.AxisListType.X)
            s1 = singles.tile([1, 1], f32)
            nc.vector.reciprocal(out=s1[:], in_=mx[:])
            s2 = singles.tile([1, 1], f32)
            nc.vector.tensor_tensor(
                out=s2[:], in0=s1[:], in1=s1[:], op=mybir.AluOpType.mult
            )
            sb = singles.tile([PT, 1], f32)
            nc.gpsimd.partition_broadcast(out_ap=sb[:], in_ap=s1[:])
            s2b = singles.tile([PT, 1], f32)
            nc.gpsimd.partition_broadcast(out_ap=s2b[:], in_ap=s2[:])
            b2 = singles.tile([PT, 1], f32)
            nc.vector.tensor_scalar(
                out=b2[:], in0=s2b[:], scalar1=SQRT_BIAS, scalar2=None,
                op0=mybir.AluOpType.mult,
            )

            # ---- phase 2: scale + store ----
            # Set A: in-place DVE scale per chunk, DMA on the SP queue.
            for k in sorted(SET_A):
                w = WIDTHS[k]
                for ch0 in range(0, w, CT):
                    cw = min(CT, w - ch0)
                    uo = UOFF[k] + ch0
                    nc.vector.tensor_scalar(
                        out=u_store[:, uo : uo + cw],
                        in0=u_store[:, uo : uo + cw],
                        scalar1=sb[:],
                        scalar2=None,
                        op0=mybir.AluOpType.mult,
                    )
                    nc.sync.dma_start(
                        out=out[:, OFF[k] + ch0 : OFF[k] + ch0 + cw],
                        in_=u_store[:, uo : uo + cw],
                    )
            # Set B: PE recompute (prefetched during the collective), fused
            # ACT sqrt-scale, DMA on the Pool SWDGE queue.
            for k in SEQ2_B:
                w = WIDTHS[k]
                c0 = N - w
                pos = POS_OF[k]
                for ch0 in range(0, w, CT):
                    cw = min(CT, w - ch0)
                    ps = psp.tile([PT, CT], f32, tag="ps")
                    for j0 in range(0, cw, FT):
                        jw = min(FT, cw - j0)
                        nc.tensor.matmul(
                            ps[:, j0 : j0 + jw],
                            kxm_s[:, pos * PT : (pos + 1) * PT],
                            kxn_s[:, c0 + ch0 + j0 : c0 + ch0 + j0 + jw],
                            start=True,
                            stop=True,
                        )
                    st = stage.tile([PT, CT], f16, tag="st")
                    nc.scalar.activation(
                        out=st[:, :cw],
                        in_=ps[:, :cw],
                        func=mybir.ActivationFunctionType.Sqrt,
                        bias=b2[:],
                        scale=s2b[:],
                    )
                    nc.gpsimd.dma_start(
                        out=out[:, OFF[k] + ch0 : OFF[k] + ch0 + cw], in_=st[:, :cw]
                    )

    nc.finalize()
    return nc


def _get_nc():
    if "nc" not in _CACHE:
        _CACHE["nc"] = _build_nc()
    return _CACHE["nc"]


def kernel(x):
    global LAST_RESULTS
    import ml_dtypes
    from concourse.bass_utils import run_bass_kernel_spmd

    bf16 = ml_dtypes.bfloat16
    x = np.asarray(x, dtype=np.float32)
    assert x.shape == (B, N, D), x.shape

    in_maps = []
    for c in range(NCORES):
        b, p = divmod(c, 2)
        xb = x[b]
        sq = (xb.astype(np.float64) ** 2).sum(-1).astype(np.float32)
        # kxn: moving operand [K, N] = [x^T; 1; sq]
        kxn = np.empty((K, N), dtype=np.float32)
        kxn[:D] = xb.T
        kxn[D] = 1.0
        kxn[D + 1] = sq
        # kxm: stationary operand [K, 16*128]; block pos <-> tile TILE_POS[pos]
        kxm = np.empty((K, NK * PT), dtype=np.float32)
        for pos, k in enumerate(TILE_POS):
            r0 = PT * (2 * k + p)
            rows = slice(r0, r0 + PT)
            kxm[:D, pos * PT : (pos + 1) * PT] = (-2.0 * xb[rows]).T
            kxm[D, pos * PT : (pos + 1) * PT] = sq[rows]
            kxm[D + 1, pos * PT : (pos + 1) * PT] = 1.0
        in_maps.append(
            {
                "kxm": np.ascontiguousarray(kxm.astype(bf16)),
                "kxn": np.ascontiguousarray(kxn.astype(bf16)),
            }
        )

    nc = _get_nc()
    res = run_bass_kernel_spmd(nc, in_maps, core_ids=list(range(NCORES)))
    LAST_RESULTS = res

    out = np.zeros((B, N, N), dtype=np.float32)
    for c in range(NCORES):
        b, p = divmod(c, 2)
        buf = np.asarray(res.results[c]["out"])
        for k in range(NK):
            w = WIDTHS[k]
            r0 = PT * (2 * k + p)
            out[b, r0 : r0 + PT, N - w :] = buf[:, OFF[k] : OFF[k] + w]
    # mirror the strict upper triangle into the lower one; diagonal = 1.0
    for b in range(B):
        m = np.triu(out[b], 1)
        out[b] = m + m.T
        np.fill_diagonal(out[b], 1.0)
    return out
